# revision 27
# baseline (speedup 1.0000x reference)
"""Trainium2 Bass kernel: multi-head attention block (B=4, N=2048, C=1024, H=16).

Sharding: 8 cores = (batch b in 0..3) x (head-group hg in 0..1, 8 heads each).
Each core computes qkv for its heads, full attention for its heads over its
batch, and a partial projection (its 512 rows of W_proj). Host sums the two
partials per batch and adds b_proj (+ bv @ W_proj, see below).

Device layout choices (all matmuls bf16 inputs, fp32 PSUM accumulate):
  - q,k produced in transposed layout qkT[dim, token] so S^T = k^T-chunks.T @ q^T
    needs no on-chip transposes.
  - v produced in natural layout [token, 65*h] with a trailing ones column per
    head, so the O matmul lhsT=[v|ones] gives row 64 = softmax denominator and
    rows 0..63 = unnormalized o^T in one PSUM accumulation chain.
  - v bias is NOT added on device: rows of the normalized attention matrix sum
    to one, so the bv contribution to the final output is the token-independent
    row bv @ W_proj, folded into the host-side b_proj add (exact).
  - exp on ScalarE with fused scale=1/8; no max subtraction (logits bounded).
  - head PAIRS: the two heads' S matmuls sit at base partitions 0/64 (distinct
    PE row groups) and issue back-to-back, so they run concurrently.
  - schedule: the Scalar engine (exp) is saturated during attention, so ALL
    other PE work (v chunks, later pairs' qk chains, output projection) is
    emitted as per-step filler inside the attention j-loops instead of in
    serial pre/post phases. Attention starts after just 16 qk matmuls.
"""

import os
import sys
from collections import deque
from contextlib import ExitStack

import numpy as np
import ml_dtypes

import concourse.bass as bass
import concourse.tile as tile
from concourse import bacc, mybir
from concourse.bass import ds, ts
from concourse.bass_utils import run_bass_kernel_spmd

try:  # without the NTFF hook module, a stray BASS_TRACE=1 would crash the run
    from antenv.axon_hooks import get_axon_ntff_profile_hook  # noqa: F401
except ImportError:
    os.environ.setdefault("BASS_NEVER_TRACE", "1")

BF16 = mybir.dt.bfloat16
F32 = mybir.dt.float32
NP_BF16 = ml_dtypes.bfloat16

B, N, C = 4, 2048, 1024
H, D = 16, 64
HPC = 8            # heads per core
CD = HPC * D       # 512 local qkv dims per core
E = D + 1          # 65: 64 v dims + ones column

LAST_RESULTS = None  # stash for test harness (exec_time_ns, trace paths)


def _build_program():
    nc = bacc.Bacc("TRN2", target_bir_lowering=False, debug=False)

    xT_d = nc.dram_tensor("xT", [C, N], BF16, kind="ExternalInput").ap()
    wqk_d = nc.dram_tensor("wqk", [C, 2 * CD], BF16, kind="ExternalInput").ap()
    wv_d = nc.dram_tensor("wv", [C, CD], BF16, kind="ExternalInput").ap()
    bqk_d = nc.dram_tensor("bqk", [128, 8], F32, kind="ExternalInput").ap()
    wp_d = nc.dram_tensor("wp", [CD, C], BF16, kind="ExternalInput").ap()
    out_d = nc.dram_tensor("out", [N, C], BF16, kind="ExternalOutput").ap()

    with tile.TileContext(nc) as tc, ExitStack() as ctx:
        singles = ctx.enter_context(tc.tile_pool(name="singles", bufs=1))
        ps_pool = ctx.enter_context(tc.tile_pool(name="ps", bufs=2, space="PSUM"))
        st_pool = ctx.enter_context(tc.tile_pool(name="stp", bufs=2, space="PSUM"))
        ot_pool = ctx.enter_context(tc.tile_pool(name="ot", bufs=2, space="PSUM"))
        exp_pool = ctx.enter_context(tc.tile_pool(name="expp", bufs=22))
        misc = ctx.enter_context(tc.tile_pool(name="misc", bufs=4))
        ob_pool = ctx.enter_context(tc.tile_pool(name="ob", bufs=3))

        # Persistent SBUF tensors, chunk-major: [partition, chunk, free].
        xT_sb = singles.tile([128, 8, N], BF16)        # x^T   [c, token]
        wqk_sb = singles.tile([128, 8, 2 * CD], BF16)  # W_qk  [c, m]
        wv_sb = singles.tile([128, 8, CD], BF16)       # W_v   [c, n]
        bqk_sb = singles.tile([128, 8], F32)
        wp_sb = singles.tile([128, 4, C], BF16)        # W_proj [hd, n]
        qkT_sb = singles.tile([128, 8, N], BF16)       # chunks 0..3 = q, 4..7 = k
        v_sb = singles.tile([128, 16, HPC * E], BF16)  # [token-in-chunk, tchunk, h*(64+1)]
        oT_sb = singles.tile([128, 4, N], BF16)        # o^T, proj lhsT layout

        # DMA in need-by order; x is split per (token-block, chunk) so the
        # early transfers spread over parallel DMA queues (a single transfer
        # is ~22.5 GB/s on one engine). Weight m-blocks stay whole — their
        # column-sliced APs already fragment into small descriptors, and
        # finer splits only add per-descriptor overhead.
        wqk_r = wqk_d.rearrange("(c p) m -> p c m", p=128)
        xT_r = xT_d.rearrange("(c p) t -> p c t", p=128)
        nc.sync.dma_start(bqk_sb, bqk_d)
        for m in (0, 4):
            nc.sync.dma_start(wqk_sb[:, :, ts(m, 128)], wqk_r[:, :, ts(m, 128)])
        for kc in range(8):
            nc.sync.dma_start(xT_sb[:, kc, ts(0, 512)], xT_r[:, kc, ts(0, 512)])
        nc.sync.dma_start(wv_sb, wv_d.rearrange("(c p) m -> p c m", p=128))
        for t4 in range(1, 4):
            for kc in range(8):
                nc.sync.dma_start(xT_sb[:, kc, ts(t4, 512)], xT_r[:, kc, ts(t4, 512)])
        for m in (1, 5, 2, 6, 3, 7):
            nc.sync.dma_start(wqk_sb[:, :, ts(m, 128)], wqk_r[:, :, ts(m, 128)])
        nc.sync.dma_start(wp_sb, wp_d.rearrange("(c p) n -> p c n", p=128))

        # qk chunk m, query-block i4q, as a list of single-matmul thunks so
        # they can be spread through attention j-loops as PE filler.
        def qk_block_thunks(m, i4q):
            thunks = []
            hold = {}

            def mk(kc, m=m, i4q=i4q, hold=hold):
                def f():
                    if kc == 0:
                        hold["ps"] = ps_pool.tile([128, 512], F32, tag="big", name="qkps")
                    nc.tensor.matmul(
                        hold["ps"],
                        wqk_sb[:, kc, ts(m, 128)],
                        xT_sb[:, kc, ds(i4q * 512, 512)],
                        start=(kc == 0),
                        stop=(kc == 7),
                    )
                    if kc == 7:
                        nc.vector.tensor_scalar_add(
                            qkT_sb[:, m, ds(i4q * 512, 512)],
                            hold["ps"],
                            bqk_sb[:, ds(m, 1)],
                        )

                return f

            for kc in range(8):
                thunks.append(mk(kc))
            return thunks

        # v chunk t (no bias matmul; bv is folded into the host b_proj add).
        def v_chunk_thunks(t):
            thunks = []
            hold = {}

            def mk(kc, t=t, hold=hold):
                def f():
                    if kc == 0:
                        hold["ps"] = ps_pool.tile([128, 512], F32, tag="big", name="vps")
                    nc.tensor.matmul(
                        hold["ps"],
                        xT_sb[:, kc, ts(t, 128)],
                        wv_sb[:, kc, :],
                        start=(kc == 0),
                        stop=(kc == 7),
                    )
                    if kc == 7:
                        vv = v_sb[:, t].rearrange("p (h e) -> p h e", e=E)
                        nc.vector.tensor_copy(
                            vv[:, :, 0:D], hold["ps"].rearrange("p (h d) -> p h d", d=D)
                        )
                        nc.vector.memset(vv[:, :, D : D + 1], 1.0)

                return f

            for kc in range(8):
                thunks.append(mk(kc))
            return thunks

        # Output projection for token chunk t, output half nh (as thunks).
        def proj_thunks(t, nh):
            thunks = []
            hold = {}

            def mk(hc, t=t, nh=nh, hold=hold):
                def f():
                    if hc == 0:
                        hold["pp"] = ps_pool.tile([128, 512], F32, tag="big", name="pjps")
                    nc.tensor.matmul(
                        hold["pp"],
                        oT_sb[:, hc, ts(t, 128)],
                        wp_sb[:, hc, ds(nh * 512, 512)],
                        start=(hc == 0),
                        stop=(hc == 3),
                    )
                    if hc == 3:
                        # bf16 partials (summed in fp32 on the host) halve the
                        # out-DMA; one DMA per chain keeps the in-order Sync
                        # queue clear for the normalize DMAs.
                        ob = ob_pool.tile([128, 512], BF16)
                        nc.vector.tensor_copy(ob, hold["pp"])
                        nc.sync.dma_start(out_d[ts(t, 128), ds(nh * 512, 512)], ob)

                return f

            for hc in range(4):
                thunks.append(mk(hc))
            return thunks

        # Two-level filler queue of (chain_key, thunk): `hi` holds qk chains
        # in need-by order (consumed first — an attention window emitted
        # before its qk producers would stall the Scalar engine), `lo` holds
        # projection partials (pure slack filler, no deadline until drain).
        # require(key) drains `hi` until that chain has fully run.
        filler_hi = deque()
        filler_lo = deque()
        pending = {}

        def add_chain(key, thunks, lo=False):
            pending[key] = pending.get(key, 0) + len(thunks)
            (filler_lo if lo else filler_hi).extend((key, th) for th in thunks)

        def _run_one():
            key, th = (filler_hi or filler_lo).popleft()
            th()
            pending[key] -= 1

        def pump(k):
            for _ in range(k):
                if not (filler_hi or filler_lo):
                    return
                _run_one()

        def require(key):
            while pending.get(key, 0) > 0:
                key2, th = filler_hi.popleft()
                th()
                pending[key2] -= 1

        def drain():
            while filler_hi or filler_lo:
                _run_one()

        def softmax_normalize(p, i4, hp_, ot):
            i0 = i4 * 512
            # Copy the whole accumulator out first: frees the PSUM slot
            # fast; same DVE cost as one row (partitions are parallel).
            otc = misc.tile([65, 512], F32, tag="otc")
            nc.vector.tensor_copy(otc, ot)
            # Softmax denominators: lane-scatter so reciprocal runs on
            # 128 lanes x 4 elems instead of 1 lane x 512 (DVE divide
            # is ~8 cycles/elem serial per lane).
            s_t = misc.tile([128, 4], F32, tag="sct")
            nc.sync.dma_start(s_t, otc[64:65])
            r_t = misc.tile([128, 4], F32, tag="rct")
            nc.vector.reciprocal(r_t, s_t)
            rec0 = misc.tile([1, 512], F32, tag="rec0")
            nc.sync.dma_start(rec0, r_t)
            recb = misc.tile([64, 512], F32, tag="recb")
            nc.gpsimd.partition_broadcast(recb, rec0)
            if hp_ == 0:
                # head A lives on partitions 0..63: write oT in place
                nc.vector.tensor_mul(oT_sb[0:64, p, ds(i0, 512)], otc[0:64], recb)
            else:
                tmp = misc.tile([64, 512], BF16, tag="tmp")
                nc.vector.tensor_mul(tmp, otc[0:64], recb)
                nc.sync.dma_start(oT_sb[64:128, p, ds(i0, 512)], tmp)

        # Pre-attention: q/k chunks for (pair 0, block 0) gate the first S
        # matmul. (v chains must NOT be emitted here: the list scheduler uses
        # emission order as priority, and a DMA-stalled v block emitted early
        # would outrank and starve the S stream once its data lands.)
        for th in qk_block_thunks(0, 0):
            th()
        for th in qk_block_thunks(4, 0):
            th()

        # Two-pass emission. The list scheduler uses emission order as
        # priority among ready instructions, so PASS A emits only what the
        # Scalar engine's exp stream depends on (S-pairs, their qk producers,
        # and window-0's v chains — v must precede pass B's chains in ps-pool
        # slot order). PASS B emits the O accumulations, normalize, and
        # projection as lower-priority backlog that fills PE slack without
        # ever delaying the next S-pair. The 22-deep ex pool lets the O
        # stream lag the exp stream by up to ~1.3 windows.
        ex_tiles = {}
        for p in range(4):
            qA, qB = qkT_sb[0:64, p], qkT_sb[64:128, p]
            kA, kB = qkT_sb[0:64, 4 + p], qkT_sb[64:128, 4 + p]
            if p == 0:
                # Window (p0,0)'s 32 pumps cover exactly: k blocks 1..3 for
                # pair 0 (needed at j=4,8,12) then q block 1 (needed before
                # window (p0,1) is emitted).
                for i4q in (1, 2, 3):
                    add_chain(("qk", 4, i4q), qk_block_thunks(4, i4q))
                for i4q in (1, 2, 3):
                    add_chain(("qk", 0, i4q), qk_block_thunks(0, i4q))
            for i4 in range(4):
                require(("qk", p, i4))
                i0 = i4 * 512
                for j in range(16):
                    if j % 4 == 0:
                        require(("qk", 4 + p, j // 4))
                    st = st_pool.tile([128, 1024], F32, tag="st")
                    nc.tensor.matmul(
                        st[:, 0:512], kA[:, ts(j, 128)], qA[:, ds(i0, 512)],
                        start=True, stop=True,
                    )
                    nc.tensor.matmul(
                        st[:, 512:1024], kB[:, ts(j, 128)], qB[:, ds(i0, 512)],
                        start=True, stop=True,
                    )
                    ex = exp_pool.tile([128, 1024], BF16)
                    nc.scalar.activation(
                        ex, st, mybir.ActivationFunctionType.Exp, scale=float(D) ** -0.5
                    )
                    ex_tiles[(p, i4, j)] = ex
                    if p == 0 and i4 == 0:
                        # v chunk j: gated by the wv/x DMAs landing mid-window.
                        for th in v_chunk_thunks(j):
                            th()
                        pump(2)
                    else:
                        pump(1 if j % 2 else 2)
                # Queue pair p+1's chains in need-by order: first S needs q/k
                # block 0; k blocks 1..3 gate j=4/8/12 of (p+1, 0); q blocks
                # 1..3 gate windows (p+1, 1..3).
                if i4 == 0 and p < 3:
                    add_chain(("qk", p + 1, 0), qk_block_thunks(p + 1, 0))
                    for i4q in range(4):
                        add_chain(("qk", 5 + p, i4q), qk_block_thunks(5 + p, i4q))
                    for i4q in (1, 2, 3):
                        add_chain(("qk", p + 1, i4q), qk_block_thunks(p + 1, i4q))

        # PASS B: O accumulation + softmax normalize + projection.
        for p in range(4):
            hA, hB = 2 * p, 2 * p + 1
            for i4 in range(4):
                otA = ot_pool.tile([65, 512], F32, tag="ot")
                otB = ot_pool.tile([65, 512], F32, tag="ot")
                for j in range(16):
                    ex = ex_tiles.pop((p, i4, j))
                    vvj = v_sb[:, j].rearrange("p (h e) -> p h e", e=E)
                    nc.tensor.matmul(
                        otA, vvj[:, hA], ex[:, 0:512], start=(j == 0), stop=(j == 15)
                    )
                    nc.tensor.matmul(
                        otB, vvj[:, hB], ex[:, 512:1024], start=(j == 0), stop=(j == 15)
                    )
                for hp_, ot in ((0, otA), (64, otB)):
                    softmax_normalize(p, i4, hp_, ot)
                if p == 3:
                    # token block i4 now has all four pairs' oT: project it.
                    for t in range(4 * i4, 4 * i4 + 4):
                        for nh in range(2):
                            add_chain(("proj", t, nh), proj_thunks(t, nh), lo=True)

        drain()

    nc.compile()
    return nc


_PROGRAM = None


def kernel(x, W_qkv, b_qkv, W_proj, b_proj):
    global _PROGRAM, LAST_RESULTS
    x = np.asarray(x, dtype=np.float32)
    W_qkv = np.asarray(W_qkv, dtype=np.float32)
    b_qkv = np.asarray(b_qkv, dtype=np.float32)
    W_proj = np.asarray(W_proj, dtype=np.float32)
    b_proj = np.asarray(b_proj, dtype=np.float32)

    if _PROGRAM is None:
        _PROGRAM = _build_program()
    nc = _PROGRAM

    in_maps = []
    for core in range(8):
        b, hg = core // 2, core % 2
        h0 = hg * HPC
        sl = slice(h0 * D, h0 * D + CD)
        wq = W_qkv[:, 0 * C :][:, sl]
        wk = W_qkv[:, 1 * C :][:, sl]
        wv = W_qkv[:, 2 * C :][:, sl]
        bq = b_qkv[0 * C :][sl]
        bk = b_qkv[1 * C :][sl]
        in_maps.append(
            {
                "xT": np.ascontiguousarray(x[b].T).astype(NP_BF16),
                "wqk": np.concatenate([wq, wk], axis=1).astype(NP_BF16),
                "wv": np.ascontiguousarray(wv).astype(NP_BF16),
                "bqk": np.concatenate([bq, bk]).reshape(8, 128).T.astype(np.float32).copy(),
                "wp": np.ascontiguousarray(W_proj[sl, :]).astype(NP_BF16),
            }
        )

    res = run_bass_kernel_spmd(nc, in_maps, list(range(8)))
    LAST_RESULTS = res
    # rows of the normalized attention matrix sum to 1, so the v-bias term
    # contributes bv @ W_proj to every token (exact; done host-side in fp32).
    bias_row = b_proj + b_qkv[2 * C :] @ W_proj
    out = np.empty((B, N, C), dtype=np.float32)
    for b in range(B):
        out[b] = (
            res.results[2 * b]["out"].astype(np.float32)
            + res.results[2 * b + 1]["out"].astype(np.float32)
            + bias_row[None, :]
        )
    return out
